# revision 30
# baseline (speedup 1.0000x reference)
"""Trainium2 Bass kernel for a Dango-like HyperSAGNN block.

Reference computation (fp32):
  static = relu(X @ Ws.T + bs)                         # (32768, 768)
  x = X
  for l in 0..1:
      q/k/v = x @ W{q,k,v}[l].T + b{q,k,v}[l]          # per-group (16) masked MHA
      attn  = softmax over in-group, non-self keys
      out   = (attn @ v) @ Wo[l].T + bo[l]
      x     = x + beta[l] * out
  returns (static, x)

Strategy: data-parallel over groups — 8 cores x 4096 genes (256 groups).
Activations are kept feature-major ([768, genes]) in SBUF so every
projection is a dense 128-contraction with the (transposed, host-prepped)
weights stationary. Attention runs on 128-gene blocks (8 groups):
transposed scores [k, q] for a whole block come from one K=64 matmul per
head; a multiplicative block-diagonal mask applied after exp implements
in-group/non-self softmax (no max-subtraction needed: |scores/8| is
O(1)). The exp'd masked scores tile is then the stationary operand for
the attn @ V matmul against gene-major V augmented with a ones column,
which yields the unnormalized output AND the softmax denominator in one
matmul; normalization happens during the PSUM->SBUF copies (split
across the scalar and vector engines to keep both in-order queues
short). The gene-major attention output is PE-transposed back to
feature-major for the output projection. Scores pair heads of OPPOSITE
parity (head 2t in PE rows 0:64 of feature tile t, head 2t+1 in rows
64:128): the alternating row-groups let each LDWEIGHTS pull ahead of
the in-flight matmul and the even/odd matmuls run concurrently in
different sub-arrays, with even/odd scores in different PSUM banks
(mixing row-groups in one bank hangs the device). fp16 matmul inputs
(full PE rate, fp32 accumulate); the residual stream stays fp32. Host
folds beta into Wo, pre-transposes/casts all weights and X, and folds
ALL output-path constants (beta*(Wo@bv + bo) for both layers, bv via
softmax rows summing to 1) into the fp32 residual base, eliminating
the on-device bias matmuls (the l=1 projections see a ~7e-4
perturbation, far below fp8 noise).
"""

import sys

sys.path.insert(0, "/opt/trn_rl_repo")

import numpy as np

import concourse.bacc as bacc
import concourse.mybir as mybir
from concourse import tile

H = 768
NT = H // 128  # 6 feature tiles
NH = 12
HD = 64
SB = 512  # genes per superblock
NBLK = SB // 128  # attention blocks per superblock
N_CORES = 8
N_GENES = 32768
GPC = N_GENES // N_CORES  # genes per core
F16 = mybir.dt.float16
F32 = mybir.dt.float32
F8 = mybir.dt.float8e4
DR = mybir.MatmulPerfMode.DoubleRow
AF = mybir.ActivationFunctionType




def build_program(gpc: int = GPC):
    nsb = gpc // SB
    nc = bacc.Bacc(None, target_bir_lowering=False)

    xt_h = nc.dram_tensor("xt_h", [H, gpc], F16, kind="ExternalInput")
    xt_f = nc.dram_tensor("xt_f", [H, gpc], F32, kind="ExternalInput")
    wsT = nc.dram_tensor("wsT", [H, H], F16, kind="ExternalInput")
    bs = nc.dram_tensor("bs", [H, 1], F32, kind="ExternalInput")
    xt_8 = nc.dram_tensor("xt_8", [H, gpc], F8, kind="ExternalInput")
    wq8 = nc.dram_tensor("wq8", [2, 3, 128, 2 * H], F8, kind="ExternalInput")
    wk8 = nc.dram_tensor("wk8", [2, 3, 128, 2 * H], F8, kind="ExternalInput")
    wv8 = nc.dram_tensor("wv8", [2, 3, 128, 2 * H], F8, kind="ExternalInput")
    wo8 = nc.dram_tensor("wo8", [2, 3, 128, 2 * H], F8, kind="ExternalInput")
    bq = nc.dram_tensor("bq", [2, H, 1], F32, kind="ExternalInput")
    bk = nc.dram_tensor("bk", [2, H, 1], F32, kind="ExternalInput")
    mask4 = nc.dram_tensor("mask4", [128, SB], F16, kind="ExternalInput")
    ident = nc.dram_tensor("ident", [128, 128], F16, kind="ExternalInput")

    staticT = nc.dram_tensor("staticT", [H, gpc], F32, kind="ExternalOutput")
    outT = nc.dram_tensor("outT", [H, gpc], F32, kind="ExternalOutput")

    from contextlib import ExitStack

    with tile.TileContext(nc) as tc, ExitStack() as ctx:
        if True:
            wpool = ctx.enter_context(tc.tile_pool(name="wpool", bufs=1))
            xh_pool = ctx.enter_context(tc.tile_pool(name="xh", bufs=7))
            xf_pool = ctx.enter_context(tc.tile_pool(name="xf", bufs=7))
            qk_pool = ctx.enter_context(tc.tile_pool(name="qk", bufs=14))
            v_pool = ctx.enter_context(tc.tile_pool(name="vaug", bufs=6))
            ea_pool = ctx.enter_context(tc.tile_pool(name="ea", bufs=3))
            aout_pool = ctx.enter_context(tc.tile_pool(name="aout", bufs=3))
            af_pool = ctx.enter_context(tc.tile_pool(name="af", bufs=7))
            x1_pool = ctx.enter_context(tc.tile_pool(name="x1", bufs=7))
            x8_pool = ctx.enter_context(tc.tile_pool(name="x8p", bufs=8))
            res_pool = ctx.enter_context(tc.tile_pool(name="res", bufs=3))
            small_pool = ctx.enter_context(tc.tile_pool(name="small", bufs=4))
            pbig = ctx.enter_context(tc.tile_pool(name="pbig", bufs=3, space="PSUM"))
            psE = ctx.enter_context(tc.tile_pool(name="psE", bufs=1, space="PSUM"))
            psO = ctx.enter_context(tc.tile_pool(name="psO", bufs=1, space="PSUM"))
            patt = ctx.enter_context(tc.tile_pool(name="patt", bufs=2, space="PSUM"))
            ptp = ctx.enter_context(tc.tile_pool(name="ptp", bufs=1, space="PSUM"))
            # ---- resident constants / weights ----
            ws_sb = []
            wq_sb = [[], []]
            wk_sb = [[], []]
            wv_sb = [[], []]
            wo_sb = [[], []]
            for k in range(NT):
                t = wpool.tile([128, H], F16, name=f"ws{k}", tag=f"ws{k}")
                nc.sync.dma_start(t[:], wsT[k * 128 : (k + 1) * 128, :])
                ws_sb.append(t)

            def load_layer_weights():
                for l in range(2):
                    for name, dram, lst in (
                        ("wq", wq8, wq_sb),
                        ("wk", wk8, wk_sb),
                        ("wv", wv8, wv_sb),
                        ("wo", wo8, wo_sb),
                    ):
                        for kk in range(3):
                            t = wpool.tile(
                                [128, 2 * H], F8, name=f"{name}{l}{kk}", tag=f"{name}{l}{kk}"
                            )
                            nc.sync.dma_start(t[:], dram[l, kk])
                            lst[l].append(t)

            bs_t = wpool.tile([128, NT], F32, name="bs", tag="bs")
            bq_t = [wpool.tile([128, NT], F32, name=f"bq{l}", tag=f"bq{l}") for l in range(2)]
            bk_t = [wpool.tile([128, NT], F32, name=f"bk{l}", tag=f"bk{l}") for l in range(2)]
            nc.sync.dma_start(bs_t[:], bs[:, 0].rearrange("(m p) -> p m", p=128))
            for l in range(2):
                nc.sync.dma_start(bq_t[l][:], bq[l, :, 0].rearrange("(m p) -> p m", p=128))
                nc.sync.dma_start(bk_t[l][:], bk[l, :, 0].rearrange("(m p) -> p m", p=128))
            mask_t = wpool.tile([128, SB], F16, name="mask", tag="mask")
            nc.sync.dma_start(mask_t[:], mask4[:])
            ident_t = wpool.tile([128, 128], F16, name="ident", tag="ident")
            nc.sync.dma_start(ident_t[:], ident[:])

            def attn_front(l, xsrc, q_tiles, k_tiles, blk):
                """V-projection + scores + exp/mask for one 128-gene
                block.  Returns (va, a_ev, a_od) for attn_back."""
                vaug = v_pool.tile([128, NH * (HD + 1)], F16, name="vaug", tag="vaug")
                va = vaug[:].rearrange("p (h c) -> p h c", c=HD + 1)
                for half in range(2):
                    ps = pbig.tile([128, 384], F32, name="pbig", tag="pbig")
                    for kk in range(3):
                        nc.tensor.matmul(
                            ps[:],
                            xsrc[kk][:]
                            .rearrange("p (j n) -> p j n", n=SB)[
                                :, :, blk * 128 : (blk + 1) * 128
                            ],
                            wv_sb[l][kk][:]
                            .rearrange("p (j o) -> p j o", o=H)[
                                :, :, half * 384 : (half + 1) * 384
                            ],
                            start=(kk == 0),
                            stop=(kk == 2),
                            perf_mode=DR,
                        )
                    nc.vector.tensor_copy(
                        va[:, half * 6 : (half + 1) * 6, 0:HD],
                        ps[:].rearrange("p (h c) -> p h c", c=HD),
                    )
                nc.vector.memset(va[:, :, HD : HD + 1], 1.0)

                # scores: head 2t sits in PE rows 0:64 of feature tile t,
                # head 2t+1 in rows 64:128.  Alternating row-groups lets
                # the next LDWEIGHTS pull ahead of the in-flight matmul
                # (different row-group), and the even/odd matmuls run
                # concurrently in different sub-arrays.  Even and odd
                # scores MUST land in different PSUM banks (mixing
                # row-groups in one bank hangs the device).
                bs_ = blk * 128
                a_ev, a_od = [], []
                for ts in ((0, 1, 2, 3), (4, 5)):
                    w = len(ts) * 128
                    ps_e = psE.tile([128, 512], F32, name="psE", tag="psE")
                    ps_o2 = psO.tile([128, 512], F32, name="psO", tag="psO")
                    for i, t in enumerate(ts):
                        nc.tensor.matmul(
                            ps_e[:, i * 128 : (i + 1) * 128],
                            k_tiles[t][0:HD, bs_ : bs_ + 128],
                            q_tiles[t][0:HD, bs_ : bs_ + 128],
                            start=True, stop=True, tile_position=(0, 0),
                        )
                        nc.tensor.matmul(
                            ps_o2[:, i * 128 : (i + 1) * 128],
                            k_tiles[t][HD:128, bs_ : bs_ + 128],
                            q_tiles[t][HD:128, bs_ : bs_ + 128],
                            start=True, stop=True, tile_position=(HD, 0),
                        )
                    for ps_x, dest in ((ps_e, a_ev), (ps_o2, a_od)):
                        e_t = ea_pool.tile([128, 512], F16, name="e", tag="e")
                        nc.scalar.activation(
                            e_t[:, 0:w], ps_x[:, 0:w], AF.Exp, scale=0.125
                        )
                        a_t = ea_pool.tile(
                            [128, 512], F16, name="a", tag="a", bufs=6
                        )
                        nc.vector.tensor_mul(
                            a_t[:, 0:w], e_t[:, 0:w], mask_t[:, 0:w]
                        )
                        dest.append(a_t)
                return va, a_ev, a_od

            def attn_back(st, aout):
                """attn @ V (with softmax denominator via the ones column)
                + normalized copy into gene-major aout."""
                va, a_ev, a_od = st
                for t in range(6):
                    s, i = (0, t) if t < 4 else (1, t - 4)
                    ps_o = patt.tile([128, 2 * (HD + 1)], F32, name="patt", tag="patt")
                    po = ps_o[:].rearrange("p (j c) -> p j c", c=HD + 1)
                    for j, (h, a_t) in enumerate(
                        ((2 * t, a_ev[s]), (2 * t + 1, a_od[s]))
                    ):
                        nc.tensor.matmul(
                            ps_o[:, j * (HD + 1) : (j + 1) * (HD + 1)],
                            a_t[:, i * 128 : (i + 1) * 128],
                            va[:, h, :],
                            start=True, stop=True,
                        )
                    r4 = small_pool.tile([128, 2], F32, name="r4", tag="r4")
                    nc.vector.reciprocal(r4[:], po[:, :, HD])
                    # normalize-copies split across scalar/vector so the
                    # next block's exp isn't queued behind 12 copies on
                    # the in-order scalar FIFO
                    nc.scalar.activation(
                        aout[:, (2 * t) * HD : (2 * t + 1) * HD],
                        po[:, 0, 0:HD],
                        AF.Copy,
                        scale=r4[:, 0:1],
                    )
                    nc.vector.tensor_scalar_mul(
                        aout[:, (2 * t + 1) * HD : (2 * t + 2) * HD],
                        po[:, 1, 0:HD],
                        r4[:, 1:2],
                    )

            for sb in range(nsb):
                g0 = sb * SB
                xh = []
                for k in range(NT):
                    t = xh_pool.tile([128, SB], F16, name="xh", tag="xh")
                    nc.sync.dma_start(t[:], xt_h[k * 128 : (k + 1) * 128, g0 : g0 + SB])
                    xh.append(t)
                x8 = []
                for kk in range(3):
                    t = x8_pool.tile([128, 2 * SB], F8, name="x8", tag="x8")
                    for j in range(2):
                        nc.sync.dma_start(
                            t[:, j * SB : (j + 1) * SB],
                            xt_8[(2 * kk + j) * 128 : (2 * kk + j + 1) * 128, g0 : g0 + SB],
                        )
                    x8.append(t)
                if sb == 0:
                    load_layer_weights()

                # static branch
                for m in range(NT):
                    ps = pbig.tile([128, SB], F32, name="pbig", tag="pbig")
                    for k in range(NT):
                        nc.tensor.matmul(
                            ps[:],
                            ws_sb[k][:, m * 128 : (m + 1) * 128],
                            xh[k][:],
                            start=(k == 0),
                            stop=(k == NT - 1),
                        )
                    st = res_pool.tile([128, SB], F32, name="st", tag="st")
                    nc.scalar.activation(
                        st[:], ps[:], AF.Relu, bias=bs_t[:, m : m + 1], scale=1.0
                    )
                    nc.sync.dma_start(
                        staticT[m * 128 : (m + 1) * 128, g0 : g0 + SB], st[:]
                    )

                xin_8 = x8
                xin_f = None
                for l in range(2):
                    # Q, K projections (feature-major)
                    q_tiles, k_tiles = [], []
                    for dest, w_sb, b_t in (
                        (q_tiles, wq_sb[l], bq_t[l]),
                        (k_tiles, wk_sb[l], bk_t[l]),
                    ):
                        for m in range(NT):
                            ps = pbig.tile([128, SB], F32, name="pbig", tag="pbig")
                            for kk in range(3):
                                nc.tensor.matmul(
                                    ps[:],
                                    w_sb[kk][:]
                                    .rearrange("p (j o) -> p j o", o=H)[
                                        :, :, m * 128 : (m + 1) * 128
                                    ],
                                    xin_8[kk][:].rearrange("p (j n) -> p j n", n=SB),
                                    start=(kk == 0),
                                    stop=(kk == 2),
                                    perf_mode=DR,
                                )
                            qt = qk_pool.tile([128, SB], F16, name="qk", tag="qk")
                            nc.vector.tensor_scalar_add(qt[:], ps[:], b_t[:, m : m + 1])
                            dest.append(qt)

                    af8_tiles = [
                        af_pool.tile([128, 2 * SB], F8, name="af", tag="af")
                        for _ in range(3)
                    ]

                    def transp(blk, aout):
                        for t in range(NT):
                            ps_t = ptp.tile([128, 128], F16, name="ptp", tag="ptp")
                            nc.tensor.transpose(
                                ps_t[:], aout[:, t * 128 : (t + 1) * 128], ident_t[:]
                            )
                            kk, j = divmod(t, 2)
                            nc.vector.tensor_copy(
                                af8_tiles[kk][
                                    :,
                                    j * SB + blk * 128 : j * SB + (blk + 1) * 128,
                                ],
                                ps_t[:],
                            )

                    for blk in range(NBLK):
                        st0 = attn_front(l, xin_8, q_tiles, k_tiles, blk)
                        aout = aout_pool.tile([128, H], F16, name="aout", tag="aout")
                        attn_back(st0, aout)
                        transp(blk, aout)

                    if l == 0:
                        # residual base loaded late (off the startup DMA path)
                        xin_f = []
                        for k in range(NT):
                            t = xf_pool.tile([128, SB], F32, name="xf", tag="xf")
                            nc.sync.dma_start(
                                t[:], xt_f[k * 128 : (k + 1) * 128, g0 : g0 + SB]
                            )
                            xin_f.append(t)

                    # output projection (beta and all biases folded in on host)
                    new_f = []
                    for m in range(NT):
                        ps = pbig.tile([128, SB], F32, name="pbig", tag="pbig")
                        for kk in range(3):
                            nc.tensor.matmul(
                                ps[:],
                                wo_sb[l][kk][:]
                                .rearrange("p (j o) -> p j o", o=H)[
                                    :, :, m * 128 : (m + 1) * 128
                                ],
                                af8_tiles[kk][:].rearrange("p (j n) -> p j n", n=SB),
                                start=(kk == 0),
                                stop=(kk == 2),
                                perf_mode=DR,
                            )
                        if l == 0:
                            xnf = x1_pool.tile([128, SB], F32, name="x1f", tag="x1f")
                            nc.vector.tensor_add(xnf[:], ps[:], xin_f[m][:])
                            new_f.append(xnf)
                        else:
                            xo = res_pool.tile([128, SB], F32, name="xo", tag="xo")
                            nc.vector.tensor_add(xo[:], ps[:], xin_f[m][:])
                            nc.sync.dma_start(
                                outT[m * 128 : (m + 1) * 128, g0 : g0 + SB], xo[:]
                            )
                    if l == 0:
                        x8_1 = []
                        for kk in range(3):
                            t = x8_pool.tile([128, 2 * SB], F8, name="x81", tag="x8")
                            x8_1.append(t)
                        for m in range(NT):
                            kk, j = divmod(m, 2)
                            nc.scalar.activation(
                                x8_1[kk][:, j * SB : (j + 1) * SB],
                                new_f[m][:],
                                AF.Copy,
                                scale=1.0,
                            )
                        xin_f, xin_8 = new_f, x8_1

    nc.finalize()
    return nc


def host_prep(inputs: dict, core: int, gpc: int = GPC) -> dict:
    """Slice/transpose/cast inputs for one core."""
    ge = np.asarray(inputs["gene_embeddings"], np.float32)
    Ws = np.asarray(inputs["W_static"], np.float32)
    bs = np.asarray(inputs["b_static"], np.float32)
    Wq = np.asarray(inputs["Wq"], np.float32)
    bq = np.asarray(inputs["bq"], np.float32)
    Wk = np.asarray(inputs["Wk"], np.float32)
    bk = np.asarray(inputs["bk"], np.float32)
    Wv = np.asarray(inputs["Wv"], np.float32)
    bv = np.asarray(inputs["bv"], np.float32)
    Wo = np.asarray(inputs["Wo"], np.float32)
    bo = np.asarray(inputs["bo"], np.float32)
    beta = np.asarray(inputs["beta"], np.float32)

    xs = ge[core * gpc : (core + 1) * gpc].T  # [768, gpc]
    f8 = mybir.dt.np(F8)

    def pack8(WT):  # [2, H, H] (k, o) -> [2, 3, 128, 2H] DoubleRow pairs
        out = np.empty((2, 3, 128, 2 * H), f8)
        for l in range(2):
            for kk in range(3):
                for j in range(2):
                    out[l, kk, :, j * H : (j + 1) * H] = WT[
                        l, (2 * kk + j) * 128 : (2 * kk + j + 1) * 128, :
                    ].astype(f8)
        return out

    wqT_f = Wq.transpose(0, 2, 1)
    wkT_f = Wk.transpose(0, 2, 1)
    wvT_f = Wv.transpose(0, 2, 1)
    woT_f = np.stack([beta[l] * Wo[l].T for l in range(2)])
    # fold the constant part of each layer's output (beta*(Wo@bv + bo),
    # with bv folded via softmax rows summing to 1) into the fp32
    # residual base: removes the on-device bias matmuls.  The l=1
    # projections see a ~7e-4 perturbation (the l=1 constant added one
    # layer early), far below the fp8 projection noise.
    c_fold = sum(beta[l] * (Wo[l] @ bv[l] + bo[l]) for l in range(2))
    # block-diagonal (8 groups of 16) minus identity, tiled 4x
    m = np.kron(np.eye(8, dtype=np.float16), np.ones((16, 16), np.float16))
    m -= np.eye(128, dtype=np.float16)
    mask4 = np.tile(m, (1, 4))
    return {
        "xt_h": np.ascontiguousarray(xs, dtype=np.float16),
        "xt_f": np.ascontiguousarray(
            xs + c_fold[:, None], dtype=np.float32
        ),
        "wsT": np.ascontiguousarray(Ws.T, dtype=np.float16),
        "bs": bs.astype(np.float32).reshape(H, 1),
        "xt_8": np.ascontiguousarray(xs).astype(f8),
        "wq8": pack8(wqT_f),
        "wk8": pack8(wkT_f),
        "wv8": pack8(wvT_f),
        "wo8": pack8(woT_f),
        "bq": bq.astype(np.float32).reshape(2, H, 1),
        "bk": bk.astype(np.float32).reshape(2, H, 1),
        "mask4": np.ascontiguousarray(mask4),
        "ident": np.eye(128, dtype=np.float16),
    }


_CACHED = {}


def _get_program():
    if "nc" not in _CACHED:
        _CACHED["nc"] = build_program(GPC)
    return _CACHED["nc"]


def kernel(**inputs):
    from concourse.bass_utils import run_bass_kernel_spmd

    nc = _get_program()
    in_maps = [host_prep(inputs, c) for c in range(N_CORES)]
    res = run_bass_kernel_spmd(nc, in_maps, list(range(N_CORES)))
    static = np.concatenate([np.asarray(r["staticT"]).T for r in res.results], axis=0)
    x = np.concatenate([np.asarray(r["outT"]).T for r in res.results], axis=0)
    return static.astype(np.float32), x.astype(np.float32)


if __name__ == "__main__":
    nc = build_program(GPC)
    print("build ok")



# revision 32
# speedup vs baseline: 1.0046x; 1.0046x over previous
"""Trainium2 Bass kernel for a Dango-like HyperSAGNN block.

Reference computation (fp32):
  static = relu(X @ Ws.T + bs)                         # (32768, 768)
  x = X
  for l in 0..1:
      q/k/v = x @ W{q,k,v}[l].T + b{q,k,v}[l]          # per-group (16) masked MHA
      attn  = softmax over in-group, non-self keys
      out   = (attn @ v) @ Wo[l].T + bo[l]
      x     = x + beta[l] * out
  returns (static, x)

Strategy: data-parallel over groups — 8 cores x 4096 genes (256 groups).
Activations are kept feature-major ([768, genes]) in SBUF so every
projection is a dense 128-contraction with the (transposed, host-prepped)
weights stationary. Attention runs on 128-gene blocks (8 groups):
transposed scores [k, q] for a whole block come from one K=64 matmul per
head; a multiplicative block-diagonal mask applied after exp implements
in-group/non-self softmax (no max-subtraction needed: |scores/8| is
O(1)). The exp'd masked scores tile is then the stationary operand for
the attn @ V matmul against gene-major V augmented with a ones column,
which yields the unnormalized output AND the softmax denominator in one
matmul; normalization happens during the PSUM->SBUF copy on the scalar
engine. The gene-major attention output is PE-transposed back to
feature-major for the output projection. Scores pair heads of OPPOSITE
parity (head 2t in PE rows 0:64 of feature tile t, head 2t+1 in rows
64:128): the alternating row-groups let each LDWEIGHTS pull ahead of
the in-flight matmul and the even/odd matmuls run concurrently in
different sub-arrays, with even/odd scores in different PSUM banks
(mixing row-groups in one bank hangs the device). fp16 matmul inputs
(full PE rate, fp32 accumulate); the residual stream stays fp32. Host
folds beta into Wo, pre-transposes/casts all weights and X, and folds
ALL output-path constants (beta*(Wo@bv + bo) for both layers, bv via
softmax rows summing to 1) into the fp32 residual base, eliminating
the on-device bias matmuls (the l=1 projections see a ~7e-4
perturbation, far below fp8 noise).
"""

import sys

sys.path.insert(0, "/opt/trn_rl_repo")

import numpy as np

import concourse.bacc as bacc
import concourse.mybir as mybir
from concourse import tile

H = 768
NT = H // 128  # 6 feature tiles
NH = 12
HD = 64
SB = 512  # genes per superblock
NBLK = SB // 128  # attention blocks per superblock
N_CORES = 8
N_GENES = 32768
GPC = N_GENES // N_CORES  # genes per core
F16 = mybir.dt.float16
F32 = mybir.dt.float32
F8 = mybir.dt.float8e4
DR = mybir.MatmulPerfMode.DoubleRow
AF = mybir.ActivationFunctionType




def build_program(gpc: int = GPC):
    nsb = gpc // SB
    nc = bacc.Bacc(None, target_bir_lowering=False)

    xt_h = nc.dram_tensor("xt_h", [H, gpc], F16, kind="ExternalInput")
    xt_f = nc.dram_tensor("xt_f", [H, gpc], F32, kind="ExternalInput")
    wsT = nc.dram_tensor("wsT", [H, H], F16, kind="ExternalInput")
    bs = nc.dram_tensor("bs", [H, 1], F32, kind="ExternalInput")
    xt_8 = nc.dram_tensor("xt_8", [H, gpc], F8, kind="ExternalInput")
    wq8 = nc.dram_tensor("wq8", [2, 3, 128, 2 * H], F8, kind="ExternalInput")
    wk8 = nc.dram_tensor("wk8", [2, 3, 128, 2 * H], F8, kind="ExternalInput")
    wv8 = nc.dram_tensor("wv8", [2, 3, 128, 2 * H], F8, kind="ExternalInput")
    wo8 = nc.dram_tensor("wo8", [2, 3, 128, 2 * H], F8, kind="ExternalInput")
    bq = nc.dram_tensor("bq", [2, H, 1], F32, kind="ExternalInput")
    bk = nc.dram_tensor("bk", [2, H, 1], F32, kind="ExternalInput")
    mask4 = nc.dram_tensor("mask4", [128, SB], F16, kind="ExternalInput")
    ident = nc.dram_tensor("ident", [128, 128], F16, kind="ExternalInput")

    staticT = nc.dram_tensor("staticT", [H, gpc], F32, kind="ExternalOutput")
    outT = nc.dram_tensor("outT", [H, gpc], F32, kind="ExternalOutput")

    from contextlib import ExitStack

    with tile.TileContext(nc) as tc, ExitStack() as ctx:
        if True:
            wpool = ctx.enter_context(tc.tile_pool(name="wpool", bufs=1))
            xh_pool = ctx.enter_context(tc.tile_pool(name="xh", bufs=7))
            xf_pool = ctx.enter_context(tc.tile_pool(name="xf", bufs=7))
            qk_pool = ctx.enter_context(tc.tile_pool(name="qk", bufs=14))
            v_pool = ctx.enter_context(tc.tile_pool(name="vaug", bufs=6))
            ea_pool = ctx.enter_context(tc.tile_pool(name="ea", bufs=3))
            aout_pool = ctx.enter_context(tc.tile_pool(name="aout", bufs=3))
            af_pool = ctx.enter_context(tc.tile_pool(name="af", bufs=7))
            x1_pool = ctx.enter_context(tc.tile_pool(name="x1", bufs=7))
            x8_pool = ctx.enter_context(tc.tile_pool(name="x8p", bufs=8))
            res_pool = ctx.enter_context(tc.tile_pool(name="res", bufs=3))
            small_pool = ctx.enter_context(tc.tile_pool(name="small", bufs=4))
            pbig = ctx.enter_context(tc.tile_pool(name="pbig", bufs=3, space="PSUM"))
            psE = ctx.enter_context(tc.tile_pool(name="psE", bufs=1, space="PSUM"))
            psO = ctx.enter_context(tc.tile_pool(name="psO", bufs=1, space="PSUM"))
            patt = ctx.enter_context(tc.tile_pool(name="patt", bufs=2, space="PSUM"))
            ptp = ctx.enter_context(tc.tile_pool(name="ptp", bufs=1, space="PSUM"))
            # ---- resident constants / weights ----
            ws_sb = []
            wq_sb = [[], []]
            wk_sb = [[], []]
            wv_sb = [[], []]
            wo_sb = [[], []]
            for k in range(NT):
                t = wpool.tile([128, H], F16, name=f"ws{k}", tag=f"ws{k}")
                nc.sync.dma_start(t[:], wsT[k * 128 : (k + 1) * 128, :])
                ws_sb.append(t)

            def load_layer_weights():
                for l in range(2):
                    for name, dram, lst in (
                        ("wq", wq8, wq_sb),
                        ("wk", wk8, wk_sb),
                        ("wv", wv8, wv_sb),
                        ("wo", wo8, wo_sb),
                    ):
                        for kk in range(3):
                            t = wpool.tile(
                                [128, 2 * H], F8, name=f"{name}{l}{kk}", tag=f"{name}{l}{kk}"
                            )
                            nc.sync.dma_start(t[:], dram[l, kk])
                            lst[l].append(t)

            bs_t = wpool.tile([128, NT], F32, name="bs", tag="bs")
            bq_t = [wpool.tile([128, NT], F32, name=f"bq{l}", tag=f"bq{l}") for l in range(2)]
            bk_t = [wpool.tile([128, NT], F32, name=f"bk{l}", tag=f"bk{l}") for l in range(2)]
            nc.sync.dma_start(bs_t[:], bs[:, 0].rearrange("(m p) -> p m", p=128))
            for l in range(2):
                nc.sync.dma_start(bq_t[l][:], bq[l, :, 0].rearrange("(m p) -> p m", p=128))
                nc.sync.dma_start(bk_t[l][:], bk[l, :, 0].rearrange("(m p) -> p m", p=128))
            mask_t = wpool.tile([128, SB], F16, name="mask", tag="mask")
            nc.sync.dma_start(mask_t[:], mask4[:])
            ident_t = wpool.tile([128, 128], F16, name="ident", tag="ident")
            nc.sync.dma_start(ident_t[:], ident[:])

            def attn_front(l, xsrc, q_tiles, k_tiles, blk):
                """V-projection + scores + exp/mask for one 128-gene
                block.  Returns (va, a_ev, a_od) for attn_back."""
                vaug = v_pool.tile([128, NH * (HD + 1)], F16, name="vaug", tag="vaug")
                va = vaug[:].rearrange("p (h c) -> p h c", c=HD + 1)
                for half in range(2):
                    ps = pbig.tile([128, 384], F32, name="pbig", tag="pbig")
                    for kk in range(3):
                        nc.tensor.matmul(
                            ps[:],
                            xsrc[kk][:]
                            .rearrange("p (j n) -> p j n", n=SB)[
                                :, :, blk * 128 : (blk + 1) * 128
                            ],
                            wv_sb[l][kk][:]
                            .rearrange("p (j o) -> p j o", o=H)[
                                :, :, half * 384 : (half + 1) * 384
                            ],
                            start=(kk == 0),
                            stop=(kk == 2),
                            perf_mode=DR,
                        )
                    nc.vector.tensor_copy(
                        va[:, half * 6 : (half + 1) * 6, 0:HD],
                        ps[:].rearrange("p (h c) -> p h c", c=HD),
                    )
                nc.vector.memset(va[:, :, HD : HD + 1], 1.0)

                # scores: head 2t sits in PE rows 0:64 of feature tile t,
                # head 2t+1 in rows 64:128.  Alternating row-groups lets
                # the next LDWEIGHTS pull ahead of the in-flight matmul
                # (different row-group), and the even/odd matmuls run
                # concurrently in different sub-arrays.  Even and odd
                # scores MUST land in different PSUM banks (mixing
                # row-groups in one bank hangs the device).
                bs_ = blk * 128
                a_ev, a_od = [], []
                for ts in ((0, 1, 2, 3), (4, 5)):
                    w = len(ts) * 128
                    ps_e = psE.tile([128, 512], F32, name="psE", tag="psE")
                    ps_o2 = psO.tile([128, 512], F32, name="psO", tag="psO")
                    for i, t in enumerate(ts):
                        nc.tensor.matmul(
                            ps_e[:, i * 128 : (i + 1) * 128],
                            k_tiles[t][0:HD, bs_ : bs_ + 128],
                            q_tiles[t][0:HD, bs_ : bs_ + 128],
                            start=True, stop=True, tile_position=(0, 0),
                        )
                        nc.tensor.matmul(
                            ps_o2[:, i * 128 : (i + 1) * 128],
                            k_tiles[t][HD:128, bs_ : bs_ + 128],
                            q_tiles[t][HD:128, bs_ : bs_ + 128],
                            start=True, stop=True, tile_position=(HD, 0),
                        )
                    for ps_x, dest in ((ps_e, a_ev), (ps_o2, a_od)):
                        e_t = ea_pool.tile([128, 512], F16, name="e", tag="e")
                        nc.scalar.activation(
                            e_t[:, 0:w], ps_x[:, 0:w], AF.Exp, scale=0.125
                        )
                        a_t = ea_pool.tile(
                            [128, 512], F16, name="a", tag="a", bufs=6
                        )
                        nc.vector.tensor_mul(
                            a_t[:, 0:w], e_t[:, 0:w], mask_t[:, 0:w]
                        )
                        dest.append(a_t)
                return va, a_ev, a_od

            def attn_back(st, aout):
                """attn @ V (with softmax denominator via the ones column)
                + normalized copy into gene-major aout."""
                va, a_ev, a_od = st
                for t in range(6):
                    s, i = (0, t) if t < 4 else (1, t - 4)
                    ps_o = patt.tile([128, 2 * (HD + 1)], F32, name="patt", tag="patt")
                    po = ps_o[:].rearrange("p (j c) -> p j c", c=HD + 1)
                    for j, (h, a_t) in enumerate(
                        ((2 * t, a_ev[s]), (2 * t + 1, a_od[s]))
                    ):
                        nc.tensor.matmul(
                            ps_o[:, j * (HD + 1) : (j + 1) * (HD + 1)],
                            a_t[:, i * 128 : (i + 1) * 128],
                            va[:, h, :],
                            start=True, stop=True,
                        )
                    r4 = small_pool.tile([128, 2], F32, name="r4", tag="r4")
                    nc.vector.reciprocal(r4[:], po[:, :, HD])
                    for j, h in enumerate((2 * t, 2 * t + 1)):
                        nc.scalar.activation(
                            aout[:, h * HD : (h + 1) * HD],
                            po[:, j, 0:HD],
                            AF.Copy,
                            scale=r4[:, j : j + 1],
                        )

            for sb in range(nsb):
                g0 = sb * SB
                xh = []
                for k in range(NT):
                    t = xh_pool.tile([128, SB], F16, name="xh", tag="xh")
                    nc.sync.dma_start(t[:], xt_h[k * 128 : (k + 1) * 128, g0 : g0 + SB])
                    xh.append(t)
                x8 = []
                for kk in range(3):
                    t = x8_pool.tile([128, 2 * SB], F8, name="x8", tag="x8")
                    for j in range(2):
                        nc.sync.dma_start(
                            t[:, j * SB : (j + 1) * SB],
                            xt_8[(2 * kk + j) * 128 : (2 * kk + j + 1) * 128, g0 : g0 + SB],
                        )
                    x8.append(t)
                if sb == 0:
                    load_layer_weights()

                # static branch
                for m in range(NT):
                    ps = pbig.tile([128, SB], F32, name="pbig", tag="pbig")
                    for k in range(NT):
                        nc.tensor.matmul(
                            ps[:],
                            ws_sb[k][:, m * 128 : (m + 1) * 128],
                            xh[k][:],
                            start=(k == 0),
                            stop=(k == NT - 1),
                        )
                    st = res_pool.tile([128, SB], F32, name="st", tag="st")
                    nc.scalar.activation(
                        st[:], ps[:], AF.Relu, bias=bs_t[:, m : m + 1], scale=1.0
                    )
                    nc.sync.dma_start(
                        staticT[m * 128 : (m + 1) * 128, g0 : g0 + SB], st[:]
                    )

                xin_8 = x8
                xin_f = None
                for l in range(2):
                    # Q, K projections (feature-major)
                    q_tiles, k_tiles = [], []
                    for dest, w_sb, b_t in (
                        (q_tiles, wq_sb[l], bq_t[l]),
                        (k_tiles, wk_sb[l], bk_t[l]),
                    ):
                        for m in range(NT):
                            ps = pbig.tile([128, SB], F32, name="pbig", tag="pbig")
                            for kk in range(3):
                                nc.tensor.matmul(
                                    ps[:],
                                    w_sb[kk][:]
                                    .rearrange("p (j o) -> p j o", o=H)[
                                        :, :, m * 128 : (m + 1) * 128
                                    ],
                                    xin_8[kk][:].rearrange("p (j n) -> p j n", n=SB),
                                    start=(kk == 0),
                                    stop=(kk == 2),
                                    perf_mode=DR,
                                )
                            qt = qk_pool.tile([128, SB], F16, name="qk", tag="qk")
                            nc.vector.tensor_scalar_add(qt[:], ps[:], b_t[:, m : m + 1])
                            dest.append(qt)

                    af8_tiles = [
                        af_pool.tile([128, 2 * SB], F8, name="af", tag="af")
                        for _ in range(3)
                    ]

                    def transp(blk, aout):
                        for t in range(NT):
                            ps_t = ptp.tile([128, 128], F16, name="ptp", tag="ptp")
                            nc.tensor.transpose(
                                ps_t[:], aout[:, t * 128 : (t + 1) * 128], ident_t[:]
                            )
                            kk, j = divmod(t, 2)
                            nc.vector.tensor_copy(
                                af8_tiles[kk][
                                    :,
                                    j * SB + blk * 128 : j * SB + (blk + 1) * 128,
                                ],
                                ps_t[:],
                            )

                    for blk in range(NBLK):
                        st0 = attn_front(l, xin_8, q_tiles, k_tiles, blk)
                        aout = aout_pool.tile([128, H], F16, name="aout", tag="aout")
                        attn_back(st0, aout)
                        transp(blk, aout)

                    if l == 0:
                        # residual base loaded late (off the startup DMA path)
                        xin_f = []
                        for k in range(NT):
                            t = xf_pool.tile([128, SB], F32, name="xf", tag="xf")
                            nc.sync.dma_start(
                                t[:], xt_f[k * 128 : (k + 1) * 128, g0 : g0 + SB]
                            )
                            xin_f.append(t)

                    # output projection (beta and all biases folded in on host)
                    new_f = []
                    for m in range(NT):
                        ps = pbig.tile([128, SB], F32, name="pbig", tag="pbig")
                        for kk in range(3):
                            nc.tensor.matmul(
                                ps[:],
                                wo_sb[l][kk][:]
                                .rearrange("p (j o) -> p j o", o=H)[
                                    :, :, m * 128 : (m + 1) * 128
                                ],
                                af8_tiles[kk][:].rearrange("p (j n) -> p j n", n=SB),
                                start=(kk == 0),
                                stop=(kk == 2),
                                perf_mode=DR,
                            )
                        if l == 0:
                            xnf = x1_pool.tile([128, SB], F32, name="x1f", tag="x1f")
                            nc.vector.tensor_add(xnf[:], ps[:], xin_f[m][:])
                            new_f.append(xnf)
                        else:
                            xo = res_pool.tile([128, SB], F32, name="xo", tag="xo")
                            nc.vector.tensor_add(xo[:], ps[:], xin_f[m][:])
                            nc.sync.dma_start(
                                outT[m * 128 : (m + 1) * 128, g0 : g0 + SB], xo[:]
                            )
                    if l == 0:
                        x8_1 = []
                        for kk in range(3):
                            t = x8_pool.tile([128, 2 * SB], F8, name="x81", tag="x8")
                            x8_1.append(t)
                        for m in range(NT):
                            kk, j = divmod(m, 2)
                            nc.scalar.activation(
                                x8_1[kk][:, j * SB : (j + 1) * SB],
                                new_f[m][:],
                                AF.Copy,
                                scale=1.0,
                            )
                        xin_f, xin_8 = new_f, x8_1

    nc.finalize()
    return nc


def host_prep(inputs: dict, core: int, gpc: int = GPC) -> dict:
    """Slice/transpose/cast inputs for one core."""
    ge = np.asarray(inputs["gene_embeddings"], np.float32)
    Ws = np.asarray(inputs["W_static"], np.float32)
    bs = np.asarray(inputs["b_static"], np.float32)
    Wq = np.asarray(inputs["Wq"], np.float32)
    bq = np.asarray(inputs["bq"], np.float32)
    Wk = np.asarray(inputs["Wk"], np.float32)
    bk = np.asarray(inputs["bk"], np.float32)
    Wv = np.asarray(inputs["Wv"], np.float32)
    bv = np.asarray(inputs["bv"], np.float32)
    Wo = np.asarray(inputs["Wo"], np.float32)
    bo = np.asarray(inputs["bo"], np.float32)
    beta = np.asarray(inputs["beta"], np.float32)

    xs = ge[core * gpc : (core + 1) * gpc].T  # [768, gpc]
    f8 = mybir.dt.np(F8)

    def pack8(WT):  # [2, H, H] (k, o) -> [2, 3, 128, 2H] DoubleRow pairs
        out = np.empty((2, 3, 128, 2 * H), f8)
        for l in range(2):
            for kk in range(3):
                for j in range(2):
                    out[l, kk, :, j * H : (j + 1) * H] = WT[
                        l, (2 * kk + j) * 128 : (2 * kk + j + 1) * 128, :
                    ].astype(f8)
        return out

    wqT_f = Wq.transpose(0, 2, 1)
    wkT_f = Wk.transpose(0, 2, 1)
    wvT_f = Wv.transpose(0, 2, 1)
    woT_f = np.stack([beta[l] * Wo[l].T for l in range(2)])
    # fold the constant part of each layer's output (beta*(Wo@bv + bo),
    # with bv folded via softmax rows summing to 1) into the fp32
    # residual base: removes the on-device bias matmuls.  The l=1
    # projections see a ~7e-4 perturbation (the l=1 constant added one
    # layer early), far below the fp8 projection noise.
    c_fold = sum(beta[l] * (Wo[l] @ bv[l] + bo[l]) for l in range(2))
    # block-diagonal (8 groups of 16) minus identity, tiled 4x
    m = np.kron(np.eye(8, dtype=np.float16), np.ones((16, 16), np.float16))
    m -= np.eye(128, dtype=np.float16)
    mask4 = np.tile(m, (1, 4))
    return {
        "xt_h": np.ascontiguousarray(xs, dtype=np.float16),
        "xt_f": np.ascontiguousarray(
            xs + c_fold[:, None], dtype=np.float32
        ),
        "wsT": np.ascontiguousarray(Ws.T, dtype=np.float16),
        "bs": bs.astype(np.float32).reshape(H, 1),
        "xt_8": np.ascontiguousarray(xs).astype(f8),
        "wq8": pack8(wqT_f),
        "wk8": pack8(wkT_f),
        "wv8": pack8(wvT_f),
        "wo8": pack8(woT_f),
        "bq": bq.astype(np.float32).reshape(2, H, 1),
        "bk": bk.astype(np.float32).reshape(2, H, 1),
        "mask4": np.ascontiguousarray(mask4),
        "ident": np.eye(128, dtype=np.float16),
    }


_CACHED = {}


def _get_program():
    if "nc" not in _CACHED:
        _CACHED["nc"] = build_program(GPC)
    return _CACHED["nc"]


def kernel(**inputs):
    from concourse.bass_utils import run_bass_kernel_spmd

    nc = _get_program()
    in_maps = [host_prep(inputs, c) for c in range(N_CORES)]
    res = run_bass_kernel_spmd(nc, in_maps, list(range(N_CORES)))
    static = np.concatenate([np.asarray(r["staticT"]).T for r in res.results], axis=0)
    x = np.concatenate([np.asarray(r["outT"]).T for r in res.results], axis=0)
    return static.astype(np.float32), x.astype(np.float32)


if __name__ == "__main__":
    nc = build_program(GPC)
    print("build ok")



# revision 33
# speedup vs baseline: 1.0403x; 1.0355x over previous
"""Trainium2 Bass kernel for a Dango-like HyperSAGNN block.

Reference computation (fp32):
  static = relu(X @ Ws.T + bs)                         # (32768, 768)
  x = X
  for l in 0..1:
      q/k/v = x @ W{q,k,v}[l].T + b{q,k,v}[l]          # per-group (16) masked MHA
      attn  = softmax over in-group, non-self keys
      out   = (attn @ v) @ Wo[l].T + bo[l]
      x     = x + beta[l] * out
  returns (static, x)

Strategy: data-parallel over groups — 8 cores x 4096 genes (256 groups).
Activations are kept feature-major ([768, genes]) in SBUF so every
projection is a dense 128-contraction with the (transposed, host-prepped)
weights stationary. Attention runs on 128-gene blocks (8 groups):
transposed scores [k, q] for a whole block come from one K=64 matmul per
head; a multiplicative block-diagonal mask applied after exp implements
in-group/non-self softmax (no max-subtraction needed: |scores/8| is
O(1)). The exp'd masked scores tile is then the stationary operand for
the attn @ V matmul against gene-major V augmented with a ones column,
which yields the unnormalized output AND the softmax denominator in one
matmul; normalization happens during the PSUM->SBUF copy on the scalar
engine. The gene-major attention output is PE-transposed back to
feature-major for the output projection. Scores pair heads of OPPOSITE
parity (head 2t in PE rows 0:64 of feature tile t, head 2t+1 in rows
64:128): the alternating row-groups let each LDWEIGHTS pull ahead of
the in-flight matmul and the even/odd matmuls run concurrently in
different sub-arrays, with even/odd scores in different PSUM banks
(mixing row-groups in one bank hangs the device). fp16 matmul inputs
(full PE rate, fp32 accumulate); the residual stream stays fp32. Host
folds beta into Wo, pre-transposes/casts all weights and X, and folds
ALL output-path constants (beta*(Wo@bv + bo) for both layers, bv via
softmax rows summing to 1) into the fp32 residual base, eliminating
the on-device bias matmuls (the l=1 projections see a ~7e-4
perturbation, far below fp8 noise).
"""

import sys

sys.path.insert(0, "/opt/trn_rl_repo")

import numpy as np

import concourse.bacc as bacc
import concourse.mybir as mybir
from concourse import tile

H = 768
NT = H // 128  # 6 feature tiles
NH = 12
HD = 64
SB = 512  # genes per superblock
NBLK = SB // 128  # attention blocks per superblock
N_CORES = 8
N_GENES = 32768
GPC = N_GENES // N_CORES  # genes per core
F16 = mybir.dt.float16
F32 = mybir.dt.float32
F8 = mybir.dt.float8e4
DR = mybir.MatmulPerfMode.DoubleRow
AF = mybir.ActivationFunctionType




def build_program(gpc: int = GPC):
    nsb = gpc // SB
    nc = bacc.Bacc(None, target_bir_lowering=False)

    xt_h = nc.dram_tensor("xt_h", [H, gpc], F16, kind="ExternalInput")
    xt_f = nc.dram_tensor("xt_f", [H, gpc], F32, kind="ExternalInput")
    wsT = nc.dram_tensor("wsT", [H, H], F16, kind="ExternalInput")
    bs = nc.dram_tensor("bs", [H, 1], F32, kind="ExternalInput")
    xt_8 = nc.dram_tensor("xt_8", [H, gpc], F8, kind="ExternalInput")
    wq8 = nc.dram_tensor("wq8", [2, 3, 128, 2 * H], F8, kind="ExternalInput")
    wk8 = nc.dram_tensor("wk8", [2, 3, 128, 2 * H], F8, kind="ExternalInput")
    wv8 = nc.dram_tensor("wv8", [2, 3, 128, 2 * H], F8, kind="ExternalInput")
    wo8 = nc.dram_tensor("wo8", [2, 3, 128, 2 * H], F8, kind="ExternalInput")
    bq = nc.dram_tensor("bq", [2, H, 1], F32, kind="ExternalInput")
    bk = nc.dram_tensor("bk", [2, H, 1], F32, kind="ExternalInput")
    mask4 = nc.dram_tensor("mask4", [128, SB], F16, kind="ExternalInput")
    ident = nc.dram_tensor("ident", [128, 128], F16, kind="ExternalInput")

    staticT = nc.dram_tensor("staticT", [H, gpc], F32, kind="ExternalOutput")
    outT = nc.dram_tensor("outT", [H, gpc], F32, kind="ExternalOutput")

    from contextlib import ExitStack

    with tile.TileContext(nc) as tc, ExitStack() as ctx:
        if True:
            wpool = ctx.enter_context(tc.tile_pool(name="wpool", bufs=1))
            xh_pool = ctx.enter_context(tc.tile_pool(name="xh", bufs=7))
            xf_pool = ctx.enter_context(tc.tile_pool(name="xf", bufs=7))
            qk_pool = ctx.enter_context(tc.tile_pool(name="qk", bufs=14))
            v_pool = ctx.enter_context(tc.tile_pool(name="vaug", bufs=6))
            ea_pool = ctx.enter_context(tc.tile_pool(name="ea", bufs=3))
            aout_pool = ctx.enter_context(tc.tile_pool(name="aout", bufs=3))
            af_pool = ctx.enter_context(tc.tile_pool(name="af", bufs=7))
            x1_pool = ctx.enter_context(tc.tile_pool(name="x1", bufs=7))
            x8_pool = ctx.enter_context(tc.tile_pool(name="x8p", bufs=8))
            res_pool = ctx.enter_context(tc.tile_pool(name="res", bufs=3))
            small_pool = ctx.enter_context(tc.tile_pool(name="small", bufs=4))
            pbig = ctx.enter_context(tc.tile_pool(name="pbig", bufs=3, space="PSUM"))
            psE = ctx.enter_context(tc.tile_pool(name="psE", bufs=1, space="PSUM"))
            psO = ctx.enter_context(tc.tile_pool(name="psO", bufs=1, space="PSUM"))
            patt = ctx.enter_context(tc.tile_pool(name="patt", bufs=2, space="PSUM"))
            ptp = ctx.enter_context(tc.tile_pool(name="ptp", bufs=1, space="PSUM"))
            # ---- resident constants / weights ----
            ws_sb = []
            wq_sb = [[], []]
            wk_sb = [[], []]
            wv_sb = [[], []]
            wo_sb = [[], []]
            for k in range(NT):
                t = wpool.tile([128, H], F16, name=f"ws{k}", tag=f"ws{k}")
                nc.sync.dma_start(t[:], wsT[k * 128 : (k + 1) * 128, :])
                ws_sb.append(t)

            def load_layer_weights():
                for l in range(2):
                    for name, dram, lst in (
                        ("wq", wq8, wq_sb),
                        ("wk", wk8, wk_sb),
                        ("wv", wv8, wv_sb),
                        ("wo", wo8, wo_sb),
                    ):
                        for kk in range(3):
                            t = wpool.tile(
                                [128, 2 * H], F8, name=f"{name}{l}{kk}", tag=f"{name}{l}{kk}"
                            )
                            nc.sync.dma_start(t[:], dram[l, kk])
                            lst[l].append(t)

            bs_t = wpool.tile([128, NT], F32, name="bs", tag="bs")
            bq_t = [wpool.tile([128, NT], F32, name=f"bq{l}", tag=f"bq{l}") for l in range(2)]
            bk_t = [wpool.tile([128, NT], F32, name=f"bk{l}", tag=f"bk{l}") for l in range(2)]
            nc.sync.dma_start(bs_t[:], bs[:, 0].rearrange("(m p) -> p m", p=128))
            for l in range(2):
                nc.sync.dma_start(bq_t[l][:], bq[l, :, 0].rearrange("(m p) -> p m", p=128))
                nc.sync.dma_start(bk_t[l][:], bk[l, :, 0].rearrange("(m p) -> p m", p=128))
            mask_t = wpool.tile([128, SB], F16, name="mask", tag="mask")
            nc.sync.dma_start(mask_t[:], mask4[:])
            ident_t = wpool.tile([128, 128], F16, name="ident", tag="ident")
            nc.sync.dma_start(ident_t[:], ident[:])

            def attn_front(l, xsrc, q_tiles, k_tiles, blk):
                """V-projection + scores + exp/mask for one 128-gene
                block.  Returns (va, a_ev, a_od) for attn_back."""
                vaug = v_pool.tile([128, NH * (HD + 1)], F16, name="vaug", tag="vaug")
                va = vaug[:].rearrange("p (h c) -> p h c", c=HD + 1)
                for half in range(2):
                    ps = pbig.tile([128, 384], F32, name="pbig", tag="pbig")
                    for kk in range(3):
                        nc.tensor.matmul(
                            ps[:],
                            xsrc[kk][:]
                            .rearrange("p (j n) -> p j n", n=SB)[
                                :, :, blk * 128 : (blk + 1) * 128
                            ],
                            wv_sb[l][kk][:]
                            .rearrange("p (j o) -> p j o", o=H)[
                                :, :, half * 384 : (half + 1) * 384
                            ],
                            start=(kk == 0),
                            stop=(kk == 2),
                            perf_mode=DR,
                        )
                    nc.vector.tensor_copy(
                        va[:, half * 6 : (half + 1) * 6, 0:HD],
                        ps[:].rearrange("p (h c) -> p h c", c=HD),
                    )
                nc.vector.memset(va[:, :, HD : HD + 1], 1.0)

                # scores: head 2t sits in PE rows 0:64 of feature tile t,
                # head 2t+1 in rows 64:128.  Alternating row-groups lets
                # the next LDWEIGHTS pull ahead of the in-flight matmul
                # (different row-group), and the even/odd matmuls run
                # concurrently in different sub-arrays.  Even and odd
                # scores MUST land in different PSUM banks (mixing
                # row-groups in one bank hangs the device).
                bs_ = blk * 128
                a_ev, a_od = [], []
                for ts in ((0, 1, 2, 3), (4, 5)):
                    w = len(ts) * 128
                    ps_e = psE.tile([128, 512], F32, name="psE", tag="psE")
                    ps_o2 = psO.tile([128, 512], F32, name="psO", tag="psO")
                    for i, t in enumerate(ts):
                        nc.tensor.matmul(
                            ps_e[:, i * 128 : (i + 1) * 128],
                            k_tiles[t][0:HD, bs_ : bs_ + 128],
                            q_tiles[t][0:HD, bs_ : bs_ + 128],
                            start=True, stop=True, tile_position=(0, 0),
                        )
                        nc.tensor.matmul(
                            ps_o2[:, i * 128 : (i + 1) * 128],
                            k_tiles[t][HD:128, bs_ : bs_ + 128],
                            q_tiles[t][HD:128, bs_ : bs_ + 128],
                            start=True, stop=True, tile_position=(HD, 0),
                        )
                    for ps_x, dest in ((ps_e, a_ev), (ps_o2, a_od)):
                        e_t = ea_pool.tile([128, 512], F16, name="e", tag="e")
                        nc.scalar.activation(
                            e_t[:, 0:w], ps_x[:, 0:w], AF.Exp, scale=0.125
                        )
                        a_t = ea_pool.tile(
                            [128, 512], F16, name="a", tag="a", bufs=5
                        )
                        nc.vector.tensor_mul(
                            a_t[:, 0:w], e_t[:, 0:w], mask_t[:, 0:w]
                        )
                        dest.append(a_t)
                return va, a_ev, a_od

            def attn_back(st, aout):
                """attn @ V (with softmax denominator via the ones column)
                + normalized copy into gene-major aout."""
                va, a_ev, a_od = st
                for t in range(6):
                    s, i = (0, t) if t < 4 else (1, t - 4)
                    ps_o = patt.tile([128, 2 * (HD + 1)], F32, name="patt", tag="patt")
                    po = ps_o[:].rearrange("p (j c) -> p j c", c=HD + 1)
                    for j, (h, a_t) in enumerate(
                        ((2 * t, a_ev[s]), (2 * t + 1, a_od[s]))
                    ):
                        nc.tensor.matmul(
                            ps_o[:, j * (HD + 1) : (j + 1) * (HD + 1)],
                            a_t[:, i * 128 : (i + 1) * 128],
                            va[:, h, :],
                            start=True, stop=True,
                        )
                    r4 = small_pool.tile([128, 2], F32, name="r4", tag="r4")
                    nc.vector.reciprocal(r4[:], po[:, :, HD])
                    for j, h in enumerate((2 * t, 2 * t + 1)):
                        nc.scalar.activation(
                            aout[:, h * HD : (h + 1) * HD],
                            po[:, j, 0:HD],
                            AF.Copy,
                            scale=r4[:, j : j + 1],
                        )

            for sb in range(nsb):
                g0 = sb * SB
                xh = []
                for k in range(NT):
                    t = xh_pool.tile([128, SB], F16, name="xh", tag="xh")
                    nc.sync.dma_start(t[:], xt_h[k * 128 : (k + 1) * 128, g0 : g0 + SB])
                    xh.append(t)
                x8 = []
                for kk in range(3):
                    t = x8_pool.tile([128, 2 * SB], F8, name="x8", tag="x8")
                    for j in range(2):
                        nc.sync.dma_start(
                            t[:, j * SB : (j + 1) * SB],
                            xt_8[(2 * kk + j) * 128 : (2 * kk + j + 1) * 128, g0 : g0 + SB],
                        )
                    x8.append(t)
                if sb == 0:
                    load_layer_weights()

                # static branch
                for m in range(NT):
                    ps = pbig.tile([128, SB], F32, name="pbig", tag="pbig")
                    for k in range(NT):
                        nc.tensor.matmul(
                            ps[:],
                            ws_sb[k][:, m * 128 : (m + 1) * 128],
                            xh[k][:],
                            start=(k == 0),
                            stop=(k == NT - 1),
                        )
                    st = res_pool.tile([128, SB], F32, name="st", tag="st")
                    nc.scalar.activation(
                        st[:], ps[:], AF.Relu, bias=bs_t[:, m : m + 1], scale=1.0
                    )
                    nc.sync.dma_start(
                        staticT[m * 128 : (m + 1) * 128, g0 : g0 + SB], st[:]
                    )

                xin_8 = x8
                xin_f = None
                for l in range(2):
                    # Q, K projections (feature-major)
                    q_tiles, k_tiles = [], []
                    for dest, w_sb, b_t in (
                        (q_tiles, wq_sb[l], bq_t[l]),
                        (k_tiles, wk_sb[l], bk_t[l]),
                    ):
                        for m in range(NT):
                            ps = pbig.tile([128, SB], F32, name="pbig", tag="pbig")
                            for kk in range(3):
                                nc.tensor.matmul(
                                    ps[:],
                                    w_sb[kk][:]
                                    .rearrange("p (j o) -> p j o", o=H)[
                                        :, :, m * 128 : (m + 1) * 128
                                    ],
                                    xin_8[kk][:].rearrange("p (j n) -> p j n", n=SB),
                                    start=(kk == 0),
                                    stop=(kk == 2),
                                    perf_mode=DR,
                                )
                            qt = qk_pool.tile([128, SB], F16, name="qk", tag="qk")
                            nc.vector.tensor_scalar_add(qt[:], ps[:], b_t[:, m : m + 1])
                            dest.append(qt)

                    af8_tiles = [
                        af_pool.tile([128, 2 * SB], F8, name="af", tag="af")
                        for _ in range(3)
                    ]

                    def transp(blk, aout):
                        for t in range(NT):
                            ps_t = ptp.tile([128, 128], F16, name="ptp", tag="ptp")
                            nc.tensor.transpose(
                                ps_t[:], aout[:, t * 128 : (t + 1) * 128], ident_t[:]
                            )
                            kk, j = divmod(t, 2)
                            nc.vector.tensor_copy(
                                af8_tiles[kk][
                                    :,
                                    j * SB + blk * 128 : j * SB + (blk + 1) * 128,
                                ],
                                ps_t[:],
                            )

                    for blk in range(NBLK):
                        st0 = attn_front(l, xin_8, q_tiles, k_tiles, blk)
                        aout = aout_pool.tile([128, H], F16, name="aout", tag="aout")
                        attn_back(st0, aout)
                        transp(blk, aout)

                    if l == 0:
                        # residual base loaded late (off the startup DMA path)
                        xin_f = []
                        for k in range(NT):
                            t = xf_pool.tile([128, SB], F32, name="xf", tag="xf")
                            nc.sync.dma_start(
                                t[:], xt_f[k * 128 : (k + 1) * 128, g0 : g0 + SB]
                            )
                            xin_f.append(t)

                    # output projection (beta and all biases folded in on host)
                    new_f = []
                    for m in range(NT):
                        ps = pbig.tile([128, SB], F32, name="pbig", tag="pbig")
                        for kk in range(3):
                            nc.tensor.matmul(
                                ps[:],
                                wo_sb[l][kk][:]
                                .rearrange("p (j o) -> p j o", o=H)[
                                    :, :, m * 128 : (m + 1) * 128
                                ],
                                af8_tiles[kk][:].rearrange("p (j n) -> p j n", n=SB),
                                start=(kk == 0),
                                stop=(kk == 2),
                                perf_mode=DR,
                            )
                        if l == 0:
                            xnf = x1_pool.tile([128, SB], F32, name="x1f", tag="x1f")
                            nc.vector.tensor_add(xnf[:], ps[:], xin_f[m][:])
                            new_f.append(xnf)
                        else:
                            xo = res_pool.tile([128, SB], F32, name="xo", tag="xo")
                            nc.vector.tensor_add(xo[:], ps[:], xin_f[m][:])
                            nc.sync.dma_start(
                                outT[m * 128 : (m + 1) * 128, g0 : g0 + SB], xo[:]
                            )
                    if l == 0:
                        x8_1 = []
                        for kk in range(3):
                            t = x8_pool.tile([128, 2 * SB], F8, name="x81", tag="x8")
                            x8_1.append(t)
                        for m in range(NT):
                            kk, j = divmod(m, 2)
                            nc.scalar.activation(
                                x8_1[kk][:, j * SB : (j + 1) * SB],
                                new_f[m][:],
                                AF.Copy,
                                scale=1.0,
                            )
                        xin_f, xin_8 = new_f, x8_1

    nc.finalize()
    return nc


def host_prep(inputs: dict, core: int, gpc: int = GPC) -> dict:
    """Slice/transpose/cast inputs for one core."""
    ge = np.asarray(inputs["gene_embeddings"], np.float32)
    Ws = np.asarray(inputs["W_static"], np.float32)
    bs = np.asarray(inputs["b_static"], np.float32)
    Wq = np.asarray(inputs["Wq"], np.float32)
    bq = np.asarray(inputs["bq"], np.float32)
    Wk = np.asarray(inputs["Wk"], np.float32)
    bk = np.asarray(inputs["bk"], np.float32)
    Wv = np.asarray(inputs["Wv"], np.float32)
    bv = np.asarray(inputs["bv"], np.float32)
    Wo = np.asarray(inputs["Wo"], np.float32)
    bo = np.asarray(inputs["bo"], np.float32)
    beta = np.asarray(inputs["beta"], np.float32)

    xs = ge[core * gpc : (core + 1) * gpc].T  # [768, gpc]
    f8 = mybir.dt.np(F8)

    def pack8(WT):  # [2, H, H] (k, o) -> [2, 3, 128, 2H] DoubleRow pairs
        out = np.empty((2, 3, 128, 2 * H), f8)
        for l in range(2):
            for kk in range(3):
                for j in range(2):
                    out[l, kk, :, j * H : (j + 1) * H] = WT[
                        l, (2 * kk + j) * 128 : (2 * kk + j + 1) * 128, :
                    ].astype(f8)
        return out

    wqT_f = Wq.transpose(0, 2, 1)
    wkT_f = Wk.transpose(0, 2, 1)
    wvT_f = Wv.transpose(0, 2, 1)
    woT_f = np.stack([beta[l] * Wo[l].T for l in range(2)])
    # fold the constant part of each layer's output (beta*(Wo@bv + bo),
    # with bv folded via softmax rows summing to 1) into the fp32
    # residual base: removes the on-device bias matmuls.  The l=1
    # projections see a ~7e-4 perturbation (the l=1 constant added one
    # layer early), far below the fp8 projection noise.
    c_fold = sum(beta[l] * (Wo[l] @ bv[l] + bo[l]) for l in range(2))
    # block-diagonal (8 groups of 16) minus identity, tiled 4x
    m = np.kron(np.eye(8, dtype=np.float16), np.ones((16, 16), np.float16))
    m -= np.eye(128, dtype=np.float16)
    mask4 = np.tile(m, (1, 4))
    return {
        "xt_h": np.ascontiguousarray(xs, dtype=np.float16),
        "xt_f": np.ascontiguousarray(
            xs + c_fold[:, None], dtype=np.float32
        ),
        "wsT": np.ascontiguousarray(Ws.T, dtype=np.float16),
        "bs": bs.astype(np.float32).reshape(H, 1),
        "xt_8": np.ascontiguousarray(xs).astype(f8),
        "wq8": pack8(wqT_f),
        "wk8": pack8(wkT_f),
        "wv8": pack8(wvT_f),
        "wo8": pack8(woT_f),
        "bq": bq.astype(np.float32).reshape(2, H, 1),
        "bk": bk.astype(np.float32).reshape(2, H, 1),
        "mask4": np.ascontiguousarray(mask4),
        "ident": np.eye(128, dtype=np.float16),
    }


_CACHED = {}


def _get_program():
    if "nc" not in _CACHED:
        _CACHED["nc"] = build_program(GPC)
    return _CACHED["nc"]


def kernel(**inputs):
    from concourse.bass_utils import run_bass_kernel_spmd

    nc = _get_program()
    in_maps = [host_prep(inputs, c) for c in range(N_CORES)]
    res = run_bass_kernel_spmd(nc, in_maps, list(range(N_CORES)))
    static = np.concatenate([np.asarray(r["staticT"]).T for r in res.results], axis=0)
    x = np.concatenate([np.asarray(r["outT"]).T for r in res.results], axis=0)
    return static.astype(np.float32), x.astype(np.float32)


if __name__ == "__main__":
    nc = build_program(GPC)
    print("build ok")



# revision 35
# speedup vs baseline: 1.0603x; 1.0192x over previous
"""Trainium2 Bass kernel for a Dango-like HyperSAGNN block.

Reference computation (fp32):
  static = relu(X @ Ws.T + bs)                         # (32768, 768)
  x = X
  for l in 0..1:
      q/k/v = x @ W{q,k,v}[l].T + b{q,k,v}[l]          # per-group (16) masked MHA
      attn  = softmax over in-group, non-self keys
      out   = (attn @ v) @ Wo[l].T + bo[l]
      x     = x + beta[l] * out
  returns (static, x)

Strategy: data-parallel over groups — 8 cores x 4096 genes (256 groups).
Activations are kept feature-major ([768, genes]) in SBUF so every
projection is a dense 128-contraction with the (transposed, host-prepped)
weights stationary. Attention runs on 128-gene blocks (8 groups):
transposed scores [k, q] for a whole block come from one K=64 matmul per
head; a multiplicative block-diagonal mask applied after exp implements
in-group/non-self softmax (no max-subtraction needed: |scores/8| is
O(1)). The exp'd masked scores tile is then the stationary operand for
the attn @ V matmul against gene-major V augmented with a ones column,
which yields the unnormalized output AND the softmax denominator in one
matmul; normalization happens during the PSUM->SBUF copy on the scalar
engine. The gene-major attention output is PE-transposed back to
feature-major for the output projection. Scores pair heads of OPPOSITE
parity (head 2t in PE rows 0:64 of feature tile t, head 2t+1 in rows
64:128): the alternating row-groups let each LDWEIGHTS pull ahead of
the in-flight matmul and the even/odd matmuls run concurrently in
different sub-arrays, with even/odd scores in different PSUM banks
(mixing row-groups in one bank hangs the device). fp16 matmul inputs
(full PE rate, fp32 accumulate); the residual stream stays fp32. Host
folds beta into Wo, pre-transposes/casts all weights and X, and folds
ALL output-path constants (beta*(Wo@bv + bo) for both layers, bv via
softmax rows summing to 1) into the fp32 residual base, eliminating
the on-device bias matmuls (the l=1 projections see a ~7e-4
perturbation, far below fp8 noise).
"""

import sys

sys.path.insert(0, "/opt/trn_rl_repo")

import numpy as np

import concourse.bacc as bacc
import concourse.mybir as mybir
from concourse import tile

H = 768
NT = H // 128  # 6 feature tiles
NH = 12
HD = 64
SB = 512  # genes per superblock
NBLK = SB // 128  # attention blocks per superblock
N_CORES = 8
N_GENES = 32768
GPC = N_GENES // N_CORES  # genes per core
F16 = mybir.dt.float16
F32 = mybir.dt.float32
F8 = mybir.dt.float8e4
DR = mybir.MatmulPerfMode.DoubleRow
AF = mybir.ActivationFunctionType




def build_program(gpc: int = GPC):
    nsb = gpc // SB
    nc = bacc.Bacc(None, target_bir_lowering=False)

    xt_h = nc.dram_tensor("xt_h", [H, gpc], F16, kind="ExternalInput")
    xt_f = nc.dram_tensor("xt_f", [H, gpc], F32, kind="ExternalInput")
    wsT = nc.dram_tensor("wsT", [H, H], F16, kind="ExternalInput")
    bs = nc.dram_tensor("bs", [H, 1], F32, kind="ExternalInput")
    xt_8 = nc.dram_tensor("xt_8", [H, gpc], F8, kind="ExternalInput")
    wq8 = nc.dram_tensor("wq8", [2, 3, 128, 2 * H], F8, kind="ExternalInput")
    wk8 = nc.dram_tensor("wk8", [2, 3, 128, 2 * H], F8, kind="ExternalInput")
    wv8 = nc.dram_tensor("wv8", [2, 3, 128, 2 * H], F8, kind="ExternalInput")
    wo8 = nc.dram_tensor("wo8", [2, 3, 128, 2 * H], F8, kind="ExternalInput")
    bq = nc.dram_tensor("bq", [2, H, 1], F32, kind="ExternalInput")
    bk = nc.dram_tensor("bk", [2, H, 1], F32, kind="ExternalInput")
    mask4 = nc.dram_tensor("mask4", [128, SB], F16, kind="ExternalInput")
    ident = nc.dram_tensor("ident", [128, 128], F16, kind="ExternalInput")

    staticT = nc.dram_tensor("staticT", [H, gpc], F32, kind="ExternalOutput")
    outT = nc.dram_tensor("outT", [H, gpc], F32, kind="ExternalOutput")

    from contextlib import ExitStack

    with tile.TileContext(nc) as tc, ExitStack() as ctx:
        if True:
            wpool = ctx.enter_context(tc.tile_pool(name="wpool", bufs=1))
            xh_pool = ctx.enter_context(tc.tile_pool(name="xh", bufs=7))
            xf_pool = ctx.enter_context(tc.tile_pool(name="xf", bufs=7))
            qk_pool = ctx.enter_context(tc.tile_pool(name="qk", bufs=14))
            v_pool = ctx.enter_context(tc.tile_pool(name="vaug", bufs=6))
            ea_pool = ctx.enter_context(tc.tile_pool(name="ea", bufs=3))
            aout_pool = ctx.enter_context(tc.tile_pool(name="aout", bufs=3))
            af_pool = ctx.enter_context(tc.tile_pool(name="af", bufs=7))
            x1_pool = ctx.enter_context(tc.tile_pool(name="x1", bufs=7))
            x8_pool = ctx.enter_context(tc.tile_pool(name="x8p", bufs=8))
            res_pool = ctx.enter_context(tc.tile_pool(name="res", bufs=3))
            small_pool = ctx.enter_context(tc.tile_pool(name="small", bufs=4))
            pbig = ctx.enter_context(tc.tile_pool(name="pbig", bufs=3, space="PSUM"))
            psE = ctx.enter_context(tc.tile_pool(name="psE", bufs=1, space="PSUM"))
            psO = ctx.enter_context(tc.tile_pool(name="psO", bufs=1, space="PSUM"))
            patt = ctx.enter_context(tc.tile_pool(name="patt", bufs=2, space="PSUM"))
            ptp = ctx.enter_context(tc.tile_pool(name="ptp", bufs=1, space="PSUM"))
            # ---- resident constants / weights ----
            ws_sb = []
            wq_sb = [[], []]
            wk_sb = [[], []]
            wv_sb = [[], []]
            wo_sb = [[], []]
            for k in range(NT):
                t = wpool.tile([128, H], F16, name=f"ws{k}", tag=f"ws{k}")
                nc.sync.dma_start(t[:], wsT[k * 128 : (k + 1) * 128, :])
                ws_sb.append(t)

            def load_layer_weights():
                for l in range(2):
                    for name, dram, lst in (
                        ("wq", wq8, wq_sb),
                        ("wk", wk8, wk_sb),
                        ("wv", wv8, wv_sb),
                        ("wo", wo8, wo_sb),
                    ):
                        for kk in range(3):
                            t = wpool.tile(
                                [128, 2 * H], F8, name=f"{name}{l}{kk}", tag=f"{name}{l}{kk}"
                            )
                            nc.sync.dma_start(t[:], dram[l, kk])
                            lst[l].append(t)

            bs_t = wpool.tile([128, NT], F32, name="bs", tag="bs")
            bq_t = [wpool.tile([128, NT], F32, name=f"bq{l}", tag=f"bq{l}") for l in range(2)]
            bk_t = [wpool.tile([128, NT], F32, name=f"bk{l}", tag=f"bk{l}") for l in range(2)]
            nc.sync.dma_start(bs_t[:], bs[:, 0].rearrange("(m p) -> p m", p=128))
            for l in range(2):
                nc.sync.dma_start(bq_t[l][:], bq[l, :, 0].rearrange("(m p) -> p m", p=128))
                nc.sync.dma_start(bk_t[l][:], bk[l, :, 0].rearrange("(m p) -> p m", p=128))
            mask_t = wpool.tile([128, SB], F16, name="mask", tag="mask")
            nc.sync.dma_start(mask_t[:], mask4[:])
            ident_t = wpool.tile([128, 128], F16, name="ident", tag="ident")
            nc.sync.dma_start(ident_t[:], ident[:])

            def attn_front(l, xsrc, q_tiles, k_tiles, blk):
                """Scores + exp/mask, then V-projection, for one
                128-gene block.  Scores go FIRST so the V matmuls keep
                the PE busy while exp (scalar) and mask (vector) drain —
                attn_back's matmuls then find their stationary a-tiles
                ready.  Returns (va, a_ev, a_od) for attn_back."""
                # scores: head 2t sits in PE rows 0:64 of feature tile t,
                # head 2t+1 in rows 64:128.  Alternating row-groups lets
                # the next LDWEIGHTS pull ahead of the in-flight matmul
                # (different row-group), and the even/odd matmuls run
                # concurrently in different sub-arrays.  Even and odd
                # scores MUST land in different PSUM banks (mixing
                # row-groups in one bank hangs the device).
                bs_ = blk * 128
                a_ev, a_od = [], []
                for ts in ((0, 1, 2, 3), (4, 5)):
                    w = len(ts) * 128
                    ps_e = psE.tile([128, 512], F32, name="psE", tag="psE")
                    ps_o2 = psO.tile([128, 512], F32, name="psO", tag="psO")
                    for i, t in enumerate(ts):
                        nc.tensor.matmul(
                            ps_e[:, i * 128 : (i + 1) * 128],
                            k_tiles[t][0:HD, bs_ : bs_ + 128],
                            q_tiles[t][0:HD, bs_ : bs_ + 128],
                            start=True, stop=True, tile_position=(0, 0),
                        )
                        nc.tensor.matmul(
                            ps_o2[:, i * 128 : (i + 1) * 128],
                            k_tiles[t][HD:128, bs_ : bs_ + 128],
                            q_tiles[t][HD:128, bs_ : bs_ + 128],
                            start=True, stop=True, tile_position=(HD, 0),
                        )
                    for ps_x, dest in ((ps_e, a_ev), (ps_o2, a_od)):
                        e_t = ea_pool.tile([128, 512], F16, name="e", tag="e")
                        nc.scalar.activation(
                            e_t[:, 0:w], ps_x[:, 0:w], AF.Exp, scale=0.125
                        )
                        a_t = ea_pool.tile(
                            [128, 512], F16, name="a", tag="a", bufs=5
                        )
                        nc.vector.tensor_mul(
                            a_t[:, 0:w], e_t[:, 0:w], mask_t[:, 0:w]
                        )
                        dest.append(a_t)

                # V-projection (gene-major, x8 stationary) after scores:
                # these 6 DR matmuls overlap the exp/mask chain above
                vaug = v_pool.tile([128, NH * (HD + 1)], F16, name="vaug", tag="vaug")
                va = vaug[:].rearrange("p (h c) -> p h c", c=HD + 1)
                for half in range(2):
                    ps = pbig.tile([128, 384], F32, name="pbig", tag="pbig")
                    for kk in range(3):
                        nc.tensor.matmul(
                            ps[:],
                            xsrc[kk][:]
                            .rearrange("p (j n) -> p j n", n=SB)[
                                :, :, blk * 128 : (blk + 1) * 128
                            ],
                            wv_sb[l][kk][:]
                            .rearrange("p (j o) -> p j o", o=H)[
                                :, :, half * 384 : (half + 1) * 384
                            ],
                            start=(kk == 0),
                            stop=(kk == 2),
                            perf_mode=DR,
                        )
                    nc.vector.tensor_copy(
                        va[:, half * 6 : (half + 1) * 6, 0:HD],
                        ps[:].rearrange("p (h c) -> p h c", c=HD),
                    )
                nc.vector.memset(va[:, :, HD : HD + 1], 1.0)
                return va, a_ev, a_od

            def attn_back(st, aout):
                """attn @ V (with softmax denominator via the ones column)
                + normalized copy into gene-major aout."""
                va, a_ev, a_od = st
                for t in range(6):
                    s, i = (0, t) if t < 4 else (1, t - 4)
                    ps_o = patt.tile([128, 2 * (HD + 1)], F32, name="patt", tag="patt")
                    po = ps_o[:].rearrange("p (j c) -> p j c", c=HD + 1)
                    for j, (h, a_t) in enumerate(
                        ((2 * t, a_ev[s]), (2 * t + 1, a_od[s]))
                    ):
                        nc.tensor.matmul(
                            ps_o[:, j * (HD + 1) : (j + 1) * (HD + 1)],
                            a_t[:, i * 128 : (i + 1) * 128],
                            va[:, h, :],
                            start=True, stop=True,
                        )
                    r4 = small_pool.tile([128, 2], F32, name="r4", tag="r4")
                    nc.vector.reciprocal(r4[:], po[:, :, HD])
                    for j, h in enumerate((2 * t, 2 * t + 1)):
                        nc.scalar.activation(
                            aout[:, h * HD : (h + 1) * HD],
                            po[:, j, 0:HD],
                            AF.Copy,
                            scale=r4[:, j : j + 1],
                        )

            for sb in range(nsb):
                g0 = sb * SB
                xh = []
                for k in range(NT):
                    t = xh_pool.tile([128, SB], F16, name="xh", tag="xh")
                    nc.sync.dma_start(t[:], xt_h[k * 128 : (k + 1) * 128, g0 : g0 + SB])
                    xh.append(t)
                x8 = []
                for kk in range(3):
                    t = x8_pool.tile([128, 2 * SB], F8, name="x8", tag="x8")
                    for j in range(2):
                        nc.sync.dma_start(
                            t[:, j * SB : (j + 1) * SB],
                            xt_8[(2 * kk + j) * 128 : (2 * kk + j + 1) * 128, g0 : g0 + SB],
                        )
                    x8.append(t)
                if sb == 0:
                    load_layer_weights()

                # static branch
                for m in range(NT):
                    ps = pbig.tile([128, SB], F32, name="pbig", tag="pbig")
                    for k in range(NT):
                        nc.tensor.matmul(
                            ps[:],
                            ws_sb[k][:, m * 128 : (m + 1) * 128],
                            xh[k][:],
                            start=(k == 0),
                            stop=(k == NT - 1),
                        )
                    st = res_pool.tile([128, SB], F32, name="st", tag="st")
                    nc.scalar.activation(
                        st[:], ps[:], AF.Relu, bias=bs_t[:, m : m + 1], scale=1.0
                    )
                    nc.sync.dma_start(
                        staticT[m * 128 : (m + 1) * 128, g0 : g0 + SB], st[:]
                    )

                xin_8 = x8
                xin_f = None
                for l in range(2):
                    # Q, K projections (feature-major)
                    q_tiles, k_tiles = [], []
                    for dest, w_sb, b_t in (
                        (q_tiles, wq_sb[l], bq_t[l]),
                        (k_tiles, wk_sb[l], bk_t[l]),
                    ):
                        for m in range(NT):
                            ps = pbig.tile([128, SB], F32, name="pbig", tag="pbig")
                            for kk in range(3):
                                nc.tensor.matmul(
                                    ps[:],
                                    w_sb[kk][:]
                                    .rearrange("p (j o) -> p j o", o=H)[
                                        :, :, m * 128 : (m + 1) * 128
                                    ],
                                    xin_8[kk][:].rearrange("p (j n) -> p j n", n=SB),
                                    start=(kk == 0),
                                    stop=(kk == 2),
                                    perf_mode=DR,
                                )
                            qt = qk_pool.tile([128, SB], F16, name="qk", tag="qk")
                            nc.vector.tensor_scalar_add(qt[:], ps[:], b_t[:, m : m + 1])
                            dest.append(qt)

                    af8_tiles = [
                        af_pool.tile([128, 2 * SB], F8, name="af", tag="af")
                        for _ in range(3)
                    ]

                    def transp(blk, aout):
                        for t in range(NT):
                            ps_t = ptp.tile([128, 128], F16, name="ptp", tag="ptp")
                            nc.tensor.transpose(
                                ps_t[:], aout[:, t * 128 : (t + 1) * 128], ident_t[:]
                            )
                            kk, j = divmod(t, 2)
                            nc.vector.tensor_copy(
                                af8_tiles[kk][
                                    :,
                                    j * SB + blk * 128 : j * SB + (blk + 1) * 128,
                                ],
                                ps_t[:],
                            )

                    for blk in range(NBLK):
                        st0 = attn_front(l, xin_8, q_tiles, k_tiles, blk)
                        aout = aout_pool.tile([128, H], F16, name="aout", tag="aout")
                        attn_back(st0, aout)
                        transp(blk, aout)

                    if l == 0:
                        # residual base loaded late (off the startup DMA path)
                        xin_f = []
                        for k in range(NT):
                            t = xf_pool.tile([128, SB], F32, name="xf", tag="xf")
                            nc.sync.dma_start(
                                t[:], xt_f[k * 128 : (k + 1) * 128, g0 : g0 + SB]
                            )
                            xin_f.append(t)

                    # output projection (beta and all biases folded in on host)
                    new_f = []
                    for m in range(NT):
                        ps = pbig.tile([128, SB], F32, name="pbig", tag="pbig")
                        for kk in range(3):
                            nc.tensor.matmul(
                                ps[:],
                                wo_sb[l][kk][:]
                                .rearrange("p (j o) -> p j o", o=H)[
                                    :, :, m * 128 : (m + 1) * 128
                                ],
                                af8_tiles[kk][:].rearrange("p (j n) -> p j n", n=SB),
                                start=(kk == 0),
                                stop=(kk == 2),
                                perf_mode=DR,
                            )
                        if l == 0:
                            xnf = x1_pool.tile([128, SB], F32, name="x1f", tag="x1f")
                            nc.vector.tensor_add(xnf[:], ps[:], xin_f[m][:])
                            new_f.append(xnf)
                        else:
                            xo = res_pool.tile([128, SB], F32, name="xo", tag="xo")
                            nc.vector.tensor_add(xo[:], ps[:], xin_f[m][:])
                            nc.sync.dma_start(
                                outT[m * 128 : (m + 1) * 128, g0 : g0 + SB], xo[:]
                            )
                    if l == 0:
                        x8_1 = []
                        for kk in range(3):
                            t = x8_pool.tile([128, 2 * SB], F8, name="x81", tag="x8")
                            x8_1.append(t)
                        for m in range(NT):
                            kk, j = divmod(m, 2)
                            nc.scalar.activation(
                                x8_1[kk][:, j * SB : (j + 1) * SB],
                                new_f[m][:],
                                AF.Copy,
                                scale=1.0,
                            )
                        xin_f, xin_8 = new_f, x8_1

    nc.finalize()
    return nc


def host_prep(inputs: dict, core: int, gpc: int = GPC) -> dict:
    """Slice/transpose/cast inputs for one core."""
    ge = np.asarray(inputs["gene_embeddings"], np.float32)
    Ws = np.asarray(inputs["W_static"], np.float32)
    bs = np.asarray(inputs["b_static"], np.float32)
    Wq = np.asarray(inputs["Wq"], np.float32)
    bq = np.asarray(inputs["bq"], np.float32)
    Wk = np.asarray(inputs["Wk"], np.float32)
    bk = np.asarray(inputs["bk"], np.float32)
    Wv = np.asarray(inputs["Wv"], np.float32)
    bv = np.asarray(inputs["bv"], np.float32)
    Wo = np.asarray(inputs["Wo"], np.float32)
    bo = np.asarray(inputs["bo"], np.float32)
    beta = np.asarray(inputs["beta"], np.float32)

    xs = ge[core * gpc : (core + 1) * gpc].T  # [768, gpc]
    f8 = mybir.dt.np(F8)

    def pack8(WT):  # [2, H, H] (k, o) -> [2, 3, 128, 2H] DoubleRow pairs
        out = np.empty((2, 3, 128, 2 * H), f8)
        for l in range(2):
            for kk in range(3):
                for j in range(2):
                    out[l, kk, :, j * H : (j + 1) * H] = WT[
                        l, (2 * kk + j) * 128 : (2 * kk + j + 1) * 128, :
                    ].astype(f8)
        return out

    wqT_f = Wq.transpose(0, 2, 1)
    wkT_f = Wk.transpose(0, 2, 1)
    wvT_f = Wv.transpose(0, 2, 1)
    woT_f = np.stack([beta[l] * Wo[l].T for l in range(2)])
    # fold the constant part of each layer's output (beta*(Wo@bv + bo),
    # with bv folded via softmax rows summing to 1) into the fp32
    # residual base: removes the on-device bias matmuls.  The l=1
    # projections see a ~7e-4 perturbation (the l=1 constant added one
    # layer early), far below the fp8 projection noise.
    c_fold = sum(beta[l] * (Wo[l] @ bv[l] + bo[l]) for l in range(2))
    # block-diagonal (8 groups of 16) minus identity, tiled 4x
    m = np.kron(np.eye(8, dtype=np.float16), np.ones((16, 16), np.float16))
    m -= np.eye(128, dtype=np.float16)
    mask4 = np.tile(m, (1, 4))
    return {
        "xt_h": np.ascontiguousarray(xs, dtype=np.float16),
        "xt_f": np.ascontiguousarray(
            xs + c_fold[:, None], dtype=np.float32
        ),
        "wsT": np.ascontiguousarray(Ws.T, dtype=np.float16),
        "bs": bs.astype(np.float32).reshape(H, 1),
        "xt_8": np.ascontiguousarray(xs).astype(f8),
        "wq8": pack8(wqT_f),
        "wk8": pack8(wkT_f),
        "wv8": pack8(wvT_f),
        "wo8": pack8(woT_f),
        "bq": bq.astype(np.float32).reshape(2, H, 1),
        "bk": bk.astype(np.float32).reshape(2, H, 1),
        "mask4": np.ascontiguousarray(mask4),
        "ident": np.eye(128, dtype=np.float16),
    }


_CACHED = {}


def _get_program():
    if "nc" not in _CACHED:
        _CACHED["nc"] = build_program(GPC)
    return _CACHED["nc"]


def kernel(**inputs):
    from concourse.bass_utils import run_bass_kernel_spmd

    nc = _get_program()
    in_maps = [host_prep(inputs, c) for c in range(N_CORES)]
    res = run_bass_kernel_spmd(nc, in_maps, list(range(N_CORES)))
    static = np.concatenate([np.asarray(r["staticT"]).T for r in res.results], axis=0)
    x = np.concatenate([np.asarray(r["outT"]).T for r in res.results], axis=0)
    return static.astype(np.float32), x.astype(np.float32)


if __name__ == "__main__":
    nc = build_program(GPC)
    print("build ok")

